# revision 46
# baseline (speedup 1.0000x reference)
"""Trainium2 Bass kernel for BlockChunkedActivityRoutedNet.

Reference computation (B=4096, IN_F=4096, 8 chunks of 512, top-2 by mean|x|,
chunk-expert Linears 512->512, concat -> final Linear 1024->4096):

    xr = x.reshape(B, 8, 512)
    activities = mean(|xr|, axis=(0, 2))            # over the WHOLE batch
    i0, i1 = top2(activities)                        # descending
    h = concat(xr[:, i0] @ Wc[i0] + bc[i0], xr[:, i1] @ Wc[i1] + bc[i1])
    out = h @ W_final + b_final

Distribution: data-parallel over the batch across 8 NeuronCores (512 rows
each). Per-partition |x| partials are AllReduced ([128, 8] f32) so every
core computes identical top-2 routing; the partition reduce happens after
the collective.

The collective takes 65-95us wall (runtime barrier + mesh op + core start
skew) no matter when it is triggered, so the kernel fills that window with
SPECULATIVE work: L1 (x_c @ W_c + b_c) is computed for ALL 8 chunks while
the AllReduce runs, and the resulting hT tiles are streamed to a packed
DRAM table. After routing arrives, selecting a chunk is ONE indirect
row-gather ([128, 2048], 4KB lines), so the post-routing critical path is
just: top-2 + offsets (~3us) -> 2 gathers (~3us) -> L2 (~75us) -> tail.

Host-side prep inside kernel(): casts to bf16 and packs each core's x shard
and the chunk weights as [1024, 2048] tables

    xg[c*128 + p, kt*512 + b] = x.T[c*512 + kt*128 + p, b]
    wg[c*128 + p, kt*512 + d] = W_chunks[c, kt*128 + p, d]

so chunk c's tile is the static row-slice [c*128:(c+1)*128] with 4KB
contiguous lines per partition. hT_all uses the same layout
(ht[c*128 + p, d*512 + b] = hT[c][d][p, b]). W_final loads as 8
[128, 4096] bf16 tiles (8KB lines, full DMA rate). Output is written bf16
and upcast to f32 on the host.
"""

import numpy as np
import ml_dtypes

import concourse.bass as bass
import concourse.bacc as bacc
import concourse.mybir as mybir
from concourse.tile import TileContext
from concourse.bass_utils import run_bass_kernel_spmd
from concourse.masks import make_identity

dt = mybir.dt
P = 128

NUM_CHUNKS = 8
TOP_K = 2
IN_F = 4096
HID_F = 4096
OUT_F = 4096
B = 4096
CIN = IN_F // NUM_CHUNKS      # 512
COUT = HID_F // NUM_CHUNKS    # 512
N_CORES = 8
BS = B // N_CORES             # 512 rows per core

BT = BS // P                  # 4 batch tiles per core
KT = CIN // P                 # 4 k-tiles per selected chunk
DT_ = COUT // P               # 4 d-tiles per selected chunk
KF = TOP_K * DT_              # 8 k-tiles for the final matmul
OT = OUT_F // 512             # 8 output column tiles of 512
GW = KT * BS                  # 2048 packed-table row width

_cache = {}


def _build():
    nc = bacc.Bacc(num_devices=N_CORES, name="chunk_routed_net",
                   num_swdge_queues=4)

    xg = nc.dram_tensor("xg_shard", [NUM_CHUNKS * P, GW], dt.bfloat16,
                        kind="ExternalInput")
    wg = nc.dram_tensor("wg_chunks", [NUM_CHUNKS * P, GW], dt.bfloat16,
                        kind="ExternalInput")
    bc_t = nc.dram_tensor("b_chunks", [NUM_CHUNKS, COUT], dt.float32,
                          kind="ExternalInput")
    Wf = nc.dram_tensor("W_final", [COUT * TOP_K, OUT_F], dt.bfloat16,
                        kind="ExternalInput")
    bf = nc.dram_tensor("b_final", [1, OUT_F], dt.float32, kind="ExternalInput")
    cmix_d = nc.dram_tensor("cmix", [P, 2], dt.float32, kind="ExternalInput")
    out = nc.dram_tensor("out_shard", [BS, OUT_F], dt.bfloat16,
                         kind="ExternalOutput")

    # AllReduce buffers + speculative hT table
    cc_in = nc.dram_tensor("cc_in", [P, NUM_CHUNKS], dt.float32)
    cc_out = nc.dram_tensor("cc_out", [P, NUM_CHUNKS], dt.float32)
    ht_d = nc.dram_tensor("ht_all", [NUM_CHUNKS * P, DT_ * BS], dt.bfloat16)

    with TileContext(nc) as tc:
        with tc.tile_pool(name="consts", bufs=1) as consts, \
             tc.tile_pool(name="route", bufs=1) as route, \
             tc.tile_pool(name="xl", bufs=1) as xl_pool, \
             tc.tile_pool(name="wcp", bufs=1) as wc_pool, \
             tc.tile_pool(name="hta", bufs=4) as ht_pool, \
             tc.tile_pool(name="gath", bufs=1) as gath, \
             tc.tile_pool(name="bfinp", bufs=1) as bfinp, \
             tc.tile_pool(name="wfs", bufs=8) as wfs, \
             tc.tile_pool(name="outs", bufs=4) as outs:

            # ---------------- constants ----------------
            ones_col = consts.tile([P, 1], dt.float32)     # partition reduce
            nc.vector.memset(ones_col[:], 1.0)
            ones_k1 = consts.tile([1, P], dt.float32)      # K=1 bcast matmul
            nc.vector.memset(ones_k1[:], 1.0)
            ones_k1h = consts.tile([1, P], dt.bfloat16)    # K=1 bf16 bcast
            nc.vector.memset(ones_k1h[:], 1.0)
            ident = consts.tile([P, P], dt.float32)
            make_identity(nc, ident)
            ones_sq = consts.tile([P, P], dt.float32)  # bcast-sum matmul
            nc.vector.memset(ones_sq[:], 1.0)
            # C_Rf[p, :] = p (host-provided row iota)
            C_Rf = consts.tile([P, 2], dt.float32)
            nc.scalar.dma_start(C_Rf[:], cmix_d[:])

            with tc.tile_pool(name="ps_early", bufs=1, space="PSUM") as ps_early, \
                 tc.tile_pool(name="ps_h", bufs=2, space="PSUM") as ps_h:
                # ------------ activities (x split across two queues) -------
                actcol = route.tile([P, NUM_CHUNKS], dt.float32)
                scr = route.tile([P, GW], dt.bfloat16)  # ACT throwaway
                xls = []
                H = GW // 2
                for c in range(NUM_CHUNKS):
                    xlt = xl_pool.tile([P, GW], dt.bfloat16, tag=f"xl{c}",
                                       name=f"xl{c}")
                    nc.sync.dma_start(xlt[:, 0:H],
                                      xg[c * P:(c + 1) * P, 0:H])
                    nc.scalar.dma_start(xlt[:, H:GW],
                                        xg[c * P:(c + 1) * P, H:GW])
                    xls.append(xlt)
                for c in range(NUM_CHUNKS):
                    if c % 2 == 0:
                        nc.vector.tensor_reduce(
                            actcol[:, c:c + 1], xls[c][:],
                            axis=mybir.AxisListType.X, op=mybir.AluOpType.add,
                            apply_absolute_value=True)
                    else:
                        nc.scalar.activation(
                            scr[:], xls[c][:],
                            mybir.ActivationFunctionType.Abs,
                            accum_out=actcol[:, c:c + 1])

                # ------------ AllReduce of [128, 8] partials --------------
                nc.gpsimd.dma_start(cc_in.ap(), actcol[:])
                nc.gpsimd.collective_compute(
                    "AllReduce", mybir.AluOpType.add,
                    replica_groups=[list(range(N_CORES))],
                    ins=[cc_in.ap()], outs=[cc_out.ap()])

                # ---- work that fills the AllReduce window ----
                # biases first (tiny, must not queue behind the 8MB W_final:
                # the L1 evictions need bT)
                b_sb = route.tile([NUM_CHUNKS, COUT], dt.float32)
                nc.scalar.dma_start(b_sb[:], bc_t[:])
                bfin = bfinp.tile([1, OUT_F], dt.float32)
                nc.scalar.dma_start(bfin[:], bf[:])
                # b_chunks transpose: bT[p, d*8 + c] = bc[c, d*128 + p]
                bT = route.tile([P, DT_ * NUM_CHUNKS], dt.float32)
                for d in range(DT_):
                    ps_t = ps_early.tile([P, NUM_CHUNKS], dt.float32, tag="pst")
                    nc.tensor.transpose(
                        ps_t[:], b_sb[:, d * P:(d + 1) * P],
                        ident[0:NUM_CHUNKS, 0:NUM_CHUNKS])
                    nc.scalar.copy(bT[:, d * NUM_CHUNKS:(d + 1) * NUM_CHUNKS],
                                   ps_t[:])
                # b_final broadcast [128, 4096]
                bfin_h = bfinp.tile([1, OUT_F], dt.bfloat16)
                nc.vector.tensor_copy(bfin_h[:], bfin[:])
                bfin_bc = bfinp.tile([P, OUT_F], dt.float32)
                for o in range(OT):
                    sl = slice(o * 512, (o + 1) * 512)
                    ps_b = ps_early.tile([P, 512], dt.float32, tag="psb")
                    nc.tensor.matmul(ps_b[:], ones_k1h[:], bfin_h[:, sl],
                                     start=True, stop=True)
                    nc.vector.tensor_copy(bfin_bc[:, sl], ps_b[:])
                # chunk weights: all 8 chunk tiles, sync queue after x
                wc_t = []
                for c in range(NUM_CHUNKS):
                    w = wc_pool.tile([P, GW], dt.bfloat16, tag=f"wc{c}",
                                     name=f"wc{c}")
                    nc.sync.dma_start(w[:], wg[c * P:(c + 1) * P, :])
                    wc_t.append(w)
                # W_final: 8 x [128, 4096] bf16 (8KB lines), scalar queue
                wf_t = []
                for kf in range(KF):
                    w = wfs.tile([P, OUT_F], dt.bfloat16, tag="wf",
                                 name=f"wf{kf}")
                    nc.scalar.dma_start(w[:], Wf[kf * P:(kf + 1) * P, :])
                    wf_t.append(w)

                # ---- speculative L1 for ALL chunks, hT -> DRAM table ----
                for c in range(NUM_CHUNKS):
                    htc = ht_pool.tile([P, DT_ * BS], dt.bfloat16,
                                       tag="ht", name=f"ht{c}")
                    for d in range(DT_):
                        ph = ps_h.tile([P, BS], dt.float32, tag="ph",
                                       name=f"ph{c}_{d}")
                        for kt in range(KT):
                            base = kt * 512 + d * P
                            nc.tensor.matmul(
                                ph[:], wc_t[c][:, base:base + P],
                                xls[c][:, kt * 512:(kt + 1) * 512],
                                start=(kt == 0), stop=(kt == KT - 1))
                        dsl = slice(d * BS, (d + 1) * BS)
                        bias_col = bT[:, d * NUM_CHUNKS + c:
                                      d * NUM_CHUNKS + c + 1]
                        # evict in halves on DVE + ScalarE concurrently so
                        # the PSUM-group cadence stays matmul-bound
                        hb = BS // 2
                        nc.vector.tensor_scalar(
                            htc[:, d * BS:d * BS + hb], ph[:, 0:hb],
                            bias_col, scalar2=None,
                            op0=mybir.AluOpType.add)
                        nc.scalar.activation(
                            htc[:, d * BS + hb:(d + 1) * BS], ph[:, hb:BS],
                            mybir.ActivationFunctionType.Identity,
                            bias=bias_col)
                        # NOT on gpsimd queues: the collective's ring traffic
                        # holds those until the AllReduce completes, which
                        # would delay every hT write past the AR
                        weng = nc.sync if (c + d) % 2 == 0 else nc.scalar
                        weng.dma_start(
                            ht_d[c * P:(c + 1) * P, dsl], htc[:, dsl])

                # ------------ routing (post-AllReduce) ------------
                # tile_wait_until fences the scheduler: everything here is
                # modeled as unavailable until ~70us sim-time, so the whole
                # speculative-L1 stream lands before it on every engine
                with tc.tile_wait_until(0.07):
                    acts8 = route.tile([P, NUM_CHUNKS], dt.float32)
                    nc.sync.dma_start(acts8[:], cc_out.ap())
                    # one matmul: per-chunk sums broadcast on every partition
                    bc_ps = ps_early.tile([P, NUM_CHUNKS], dt.float32,
                                          tag="psc")
                    nc.tensor.matmul(bc_ps[:], ones_sq[:], acts8[:],
                                     start=True, stop=True)

                    # per-partition (redundant, identical) top-2 from PSUM
                    maxv = route.tile([P, NUM_CHUNKS], dt.float32)
                    maxi = route.tile([P, NUM_CHUNKS], dt.uint32)
                    nc.vector.max(maxv[:], bc_ps[:])
                    nc.vector.max_index(maxi[:], maxv[:], bc_ps[:])
                    maxi_f = route.tile([P, TOP_K], dt.float32)
                    nc.vector.tensor_copy(maxi_f[:], maxi[:, 0:TOP_K])

                    # offR[p, s] = sel_s*128 + p, fused + cast on write
                    offR = route.tile([P, TOP_K], dt.int32)
                    nc.vector.tensor_scalar(
                        offR[:], maxi_f[:], 128.0,
                        scalar2=C_Rf[:, 0:1],
                        op0=mybir.AluOpType.mult, op1=mybir.AluOpType.add)

            # ------------ gather selected hT (one row-gather per slot) -----
            g = [gath.tile([P, DT_ * BS], dt.bfloat16, tag=f"g{s}",
                           name=f"g{s}")
                 for s in range(TOP_K)]
            with tc.tile_wait_until(0.072):
                for s in range(TOP_K):
                    nc.gpsimd.indirect_dma_start(
                        out=g[s][:], out_offset=None,
                        in_=ht_d[:],
                        in_offset=bass.IndirectOffsetOnAxis(
                            ap=offR[:, s:s + 1], axis=0))

            with tc.tile_pool(name="ps_o", bufs=6, space="PSUM") as ps_o:
                # ------------ L2: out = h @ W_final + b_final --------------
                for o in range(OT):
                    osl = slice(o * 512, (o + 1) * 512)
                    for bt in range(BT):
                        po = ps_o.tile([P, 512], dt.float32, tag="po",
                                       name=f"po{o}_{bt}")
                        for kf in range(KF):
                            s, d = divmod(kf, DT_)
                            lsl = slice(d * BS + bt * P,
                                        d * BS + (bt + 1) * P)
                            nc.tensor.matmul(
                                po[:], g[s][:, lsl],
                                wf_t[kf][:, osl],
                                start=(kf == 0), stop=(kf == KF - 1))
                        ot_sb = outs.tile([P, 512], dt.bfloat16, tag="ot",
                                          name=f"ot{o}_{bt}")
                        nc.vector.tensor_tensor(
                            out=ot_sb[:], in0=po[:], in1=bfin_bc[:, osl],
                            op=mybir.AluOpType.add)
                        oeng = nc.sync if (o + bt) % 2 == 0 else nc.scalar
                        oeng.dma_start(
                            out[bt * P:(bt + 1) * P, osl], ot_sb[:])
    nc.compile()
    return nc


def _pack_table(a):
    # [8, 512, N] -> [1024, 4*N] with row (c*128+p) = a[c, kt*128+p, :] for
    # kt = 0..3 laid side by side
    n = a.shape[-1]
    return np.ascontiguousarray(
        a.reshape(NUM_CHUNKS, KT, P, n).transpose(0, 2, 1, 3)
        .reshape(NUM_CHUNKS * P, KT * n))


def kernel(x, W_chunks, b_chunks, W_final, b_final):
    bf16 = ml_dtypes.bfloat16
    x = np.asarray(x, dtype=np.float32).astype(bf16)
    W_chunks = np.asarray(W_chunks, dtype=np.float32).astype(bf16)
    W_final = np.asarray(W_final, dtype=np.float32).astype(bf16)
    b_chunks = np.ascontiguousarray(np.asarray(b_chunks, dtype=np.float32))
    b_final = np.ascontiguousarray(
        np.asarray(b_final, dtype=np.float32).reshape(1, OUT_F))

    wg = _pack_table(W_chunks)

    cmix = np.empty((P, 2), dtype=np.float32)
    cmix[:, :] = np.arange(P, dtype=np.float32)[:, None]

    if "nc" not in _cache:
        _cache["nc"] = _build()
    nc = _cache["nc"]

    in_maps = []
    for c in range(N_CORES):
        shard = x[c * BS:(c + 1) * BS]              # [512, 4096]
        xt = shard.T.reshape(NUM_CHUNKS, CIN, BS)   # [8, 512, 512]
        in_maps.append({
            "xg_shard": _pack_table(xt),
            "wg_chunks": wg,
            "b_chunks": b_chunks,
            "W_final": W_final,
            "b_final": b_final,
            "cmix": cmix,
        })

    res = run_bass_kernel_spmd(nc, in_maps, core_ids=list(range(N_CORES)))
    kernel.last_result = res
    return np.concatenate(
        [res.results[c]["out_shard"].astype(np.float32)
         for c in range(N_CORES)], axis=0)


kernel.last_result = None


# revision 47
# speedup vs baseline: 1.0473x; 1.0473x over previous
"""Trainium2 Bass kernel for BlockChunkedActivityRoutedNet.

Reference computation (B=4096, IN_F=4096, 8 chunks of 512, top-2 by mean|x|,
chunk-expert Linears 512->512, concat -> final Linear 1024->4096):

    xr = x.reshape(B, 8, 512)
    activities = mean(|xr|, axis=(0, 2))            # over the WHOLE batch
    i0, i1 = top2(activities)                        # descending
    h = concat(xr[:, i0] @ Wc[i0] + bc[i0], xr[:, i1] @ Wc[i1] + bc[i1])
    out = h @ W_final + b_final

Distribution: data-parallel over the batch across 8 NeuronCores (512 rows
each). Per-partition |x| partials are AllReduced ([128, 8] f32) so every
core computes identical top-2 routing; the partition reduce happens after
the collective.

The collective takes 65-95us wall (runtime barrier + mesh op + core start
skew) no matter when it is triggered, so the kernel fills that window with
SPECULATIVE work: L1 (x_c @ W_c + b_c) is computed for ALL 8 chunks while
the AllReduce runs, and the resulting hT tiles are streamed to a packed
DRAM table. After routing arrives, selecting a chunk is ONE indirect
row-gather ([128, 2048], 4KB lines), so the post-routing critical path is
just: top-2 + offsets (~3us) -> 2 gathers (~3us) -> L2 (~75us) -> tail.

Host-side prep inside kernel(): casts to bf16 and packs each core's x shard
and the chunk weights as [1024, 2048] tables

    xg[c*128 + p, kt*512 + b] = x.T[c*512 + kt*128 + p, b]
    wg[c*128 + p, kt*512 + d] = W_chunks[c, kt*128 + p, d]

so chunk c's tile is the static row-slice [c*128:(c+1)*128] with 4KB
contiguous lines per partition. hT_all uses the same layout
(ht[c*128 + p, d*512 + b] = hT[c][d][p, b]). W_final loads as 8
[128, 4096] bf16 tiles (8KB lines, full DMA rate). Output is written bf16
and upcast to f32 on the host.
"""

import numpy as np
import ml_dtypes

import concourse.bass as bass
import concourse.bacc as bacc
import concourse.mybir as mybir
from concourse.tile import TileContext
from concourse.bass_utils import run_bass_kernel_spmd
from concourse.masks import make_identity

dt = mybir.dt
P = 128

NUM_CHUNKS = 8
TOP_K = 2
IN_F = 4096
HID_F = 4096
OUT_F = 4096
B = 4096
CIN = IN_F // NUM_CHUNKS      # 512
COUT = HID_F // NUM_CHUNKS    # 512
N_CORES = 8
BS = B // N_CORES             # 512 rows per core

BT = BS // P                  # 4 batch tiles per core
KT = CIN // P                 # 4 k-tiles per selected chunk
DT_ = COUT // P               # 4 d-tiles per selected chunk
KF = TOP_K * DT_              # 8 k-tiles for the final matmul
OT = OUT_F // 512             # 8 output column tiles of 512
GW = KT * BS                  # 2048 packed-table row width

_cache = {}


def _build():
    nc = bacc.Bacc(num_devices=N_CORES, name="chunk_routed_net",
                   num_swdge_queues=4)

    xg = nc.dram_tensor("xg_shard", [NUM_CHUNKS * P, GW], dt.bfloat16,
                        kind="ExternalInput")
    wg = nc.dram_tensor("wg_chunks", [NUM_CHUNKS * P, GW], dt.bfloat16,
                        kind="ExternalInput")
    bc_t = nc.dram_tensor("b_chunks", [NUM_CHUNKS, COUT], dt.float32,
                          kind="ExternalInput")
    Wf = nc.dram_tensor("W_final", [COUT * TOP_K, OUT_F], dt.bfloat16,
                        kind="ExternalInput")
    bf = nc.dram_tensor("b_final", [1, OUT_F], dt.float32, kind="ExternalInput")
    cmix_d = nc.dram_tensor("cmix", [P, 2], dt.float32, kind="ExternalInput")
    out = nc.dram_tensor("out_shard", [BS, OUT_F], dt.bfloat16,
                         kind="ExternalOutput")

    # AllReduce buffers + speculative hT table
    cc_in = nc.dram_tensor("cc_in", [P, NUM_CHUNKS], dt.float32)
    cc_out = nc.dram_tensor("cc_out", [P, NUM_CHUNKS], dt.float32)
    ht_d = nc.dram_tensor("ht_all", [NUM_CHUNKS * P, DT_ * BS], dt.bfloat16)

    with TileContext(nc) as tc:
        with tc.tile_pool(name="consts", bufs=1) as consts, \
             tc.tile_pool(name="route", bufs=1) as route, \
             tc.tile_pool(name="xl", bufs=1) as xl_pool, \
             tc.tile_pool(name="wcp", bufs=1) as wc_pool, \
             tc.tile_pool(name="hta", bufs=4) as ht_pool, \
             tc.tile_pool(name="gath", bufs=1) as gath, \
             tc.tile_pool(name="bfinp", bufs=1) as bfinp, \
             tc.tile_pool(name="wfs", bufs=8) as wfs, \
             tc.tile_pool(name="outs", bufs=4) as outs:

            # ---------------- constants ----------------
            ones_col = consts.tile([P, 1], dt.float32)     # partition reduce
            nc.vector.memset(ones_col[:], 1.0)
            ones_k1 = consts.tile([1, P], dt.float32)      # K=1 bcast matmul
            nc.vector.memset(ones_k1[:], 1.0)
            ones_k1h = consts.tile([1, P], dt.bfloat16)    # K=1 bf16 bcast
            nc.vector.memset(ones_k1h[:], 1.0)
            ident = consts.tile([P, P], dt.float32)
            make_identity(nc, ident)
            ones_sq = consts.tile([P, P], dt.float32)  # bcast-sum matmul
            nc.vector.memset(ones_sq[:], 1.0)
            # C_Rf[p, :] = p (host-provided row iota)
            C_Rf = consts.tile([P, 2], dt.float32)
            nc.scalar.dma_start(C_Rf[:], cmix_d[:])

            with tc.tile_pool(name="ps_early", bufs=1, space="PSUM") as ps_early, \
                 tc.tile_pool(name="ps_h", bufs=2, space="PSUM") as ps_h:
                # ------------ activities (x split across two queues) -------
                actcol = route.tile([P, NUM_CHUNKS], dt.float32)
                scr = route.tile([P, GW], dt.bfloat16)  # ACT throwaway
                xls = []
                H = GW // 2
                for c in range(NUM_CHUNKS):
                    xlt = xl_pool.tile([P, GW], dt.bfloat16, tag=f"xl{c}",
                                       name=f"xl{c}")
                    nc.sync.dma_start(xlt[:, 0:H],
                                      xg[c * P:(c + 1) * P, 0:H])
                    nc.scalar.dma_start(xlt[:, H:GW],
                                        xg[c * P:(c + 1) * P, H:GW])
                    xls.append(xlt)
                for c in range(NUM_CHUNKS):
                    if c % 2 == 0:
                        nc.vector.tensor_reduce(
                            actcol[:, c:c + 1], xls[c][:],
                            axis=mybir.AxisListType.X, op=mybir.AluOpType.add,
                            apply_absolute_value=True)
                    else:
                        nc.scalar.activation(
                            scr[:], xls[c][:],
                            mybir.ActivationFunctionType.Abs,
                            accum_out=actcol[:, c:c + 1])

                # ------------ AllReduce of [128, 8] partials --------------
                nc.gpsimd.dma_start(cc_in.ap(), actcol[:])
                nc.gpsimd.collective_compute(
                    "AllReduce", mybir.AluOpType.add,
                    replica_groups=[list(range(N_CORES))],
                    ins=[cc_in.ap()], outs=[cc_out.ap()])

                # ---- work that fills the AllReduce window ----
                # biases first (tiny, must not queue behind the 8MB W_final:
                # the L1 evictions need bT)
                b_sb = route.tile([NUM_CHUNKS, COUT], dt.float32)
                nc.scalar.dma_start(b_sb[:], bc_t[:])
                bfin = bfinp.tile([1, OUT_F], dt.float32)
                nc.scalar.dma_start(bfin[:], bf[:])
                # b_chunks transpose: bT[p, d*8 + c] = bc[c, d*128 + p]
                bT = route.tile([P, DT_ * NUM_CHUNKS], dt.float32)
                for d in range(DT_):
                    ps_t = ps_early.tile([P, NUM_CHUNKS], dt.float32, tag="pst")
                    nc.tensor.transpose(
                        ps_t[:], b_sb[:, d * P:(d + 1) * P],
                        ident[0:NUM_CHUNKS, 0:NUM_CHUNKS])
                    nc.scalar.copy(bT[:, d * NUM_CHUNKS:(d + 1) * NUM_CHUNKS],
                                   ps_t[:])
                # b_final broadcast [128, 4096]
                bfin_h = bfinp.tile([1, OUT_F], dt.bfloat16)
                nc.vector.tensor_copy(bfin_h[:], bfin[:])
                bfin_bc = bfinp.tile([P, OUT_F], dt.float32)
                for o in range(OT):
                    sl = slice(o * 512, (o + 1) * 512)
                    ps_b = ps_early.tile([P, 512], dt.float32, tag="psb")
                    nc.tensor.matmul(ps_b[:], ones_k1h[:], bfin_h[:, sl],
                                     start=True, stop=True)
                    nc.vector.tensor_copy(bfin_bc[:, sl], ps_b[:])
                # chunk weights: all 8 chunk tiles, sync queue after x
                wc_t = []
                for c in range(NUM_CHUNKS):
                    w = wc_pool.tile([P, GW], dt.bfloat16, tag=f"wc{c}",
                                     name=f"wc{c}")
                    nc.sync.dma_start(w[:], wg[c * P:(c + 1) * P, :])
                    wc_t.append(w)
                # W_final: 8 x [128, 4096] bf16 (8KB lines), scalar queue
                wf_t = []
                for kf in range(KF):
                    w = wfs.tile([P, OUT_F], dt.bfloat16, tag="wf",
                                 name=f"wf{kf}")
                    nc.scalar.dma_start(w[:], Wf[kf * P:(kf + 1) * P, :])
                    wf_t.append(w)

                # ---- speculative L1 for ALL chunks, hT -> DRAM table ----
                for c in range(NUM_CHUNKS):
                    htc = ht_pool.tile([P, DT_ * BS], dt.bfloat16,
                                       tag="ht", name=f"ht{c}")
                    for d in range(DT_):
                        ph = ps_h.tile([P, BS], dt.float32, tag="ph",
                                       name=f"ph{c}_{d}")
                        for kt in range(KT):
                            base = kt * 512 + d * P
                            nc.tensor.matmul(
                                ph[:], wc_t[c][:, base:base + P],
                                xls[c][:, kt * 512:(kt + 1) * 512],
                                start=(kt == 0), stop=(kt == KT - 1))
                        dsl = slice(d * BS, (d + 1) * BS)
                        bias_col = bT[:, d * NUM_CHUNKS + c:
                                      d * NUM_CHUNKS + c + 1]
                        # evict in halves on DVE + ScalarE concurrently so
                        # the PSUM-group cadence stays matmul-bound
                        hb = BS // 2
                        nc.vector.tensor_scalar(
                            htc[:, d * BS:d * BS + hb], ph[:, 0:hb],
                            bias_col, scalar2=None,
                            op0=mybir.AluOpType.add)
                        nc.scalar.activation(
                            htc[:, d * BS + hb:(d + 1) * BS], ph[:, hb:BS],
                            mybir.ActivationFunctionType.Identity,
                            bias=bias_col)
                        # NOT on gpsimd queues: the collective's ring traffic
                        # holds those until the AllReduce completes, which
                        # would delay every hT write past the AR
                        weng = nc.sync if (c + d) % 2 == 0 else nc.scalar
                        weng.dma_start(
                            ht_d[c * P:(c + 1) * P, dsl], htc[:, dsl])

                # ------------ routing (post-AllReduce) ------------
                # tile_wait_until fences the scheduler: everything here is
                # modeled as unavailable until ~70us sim-time, so the whole
                # speculative-L1 stream lands before it on every engine
                with tc.tile_wait_until(0.15):
                    acts8 = route.tile([P, NUM_CHUNKS], dt.float32)
                    nc.sync.dma_start(acts8[:], cc_out.ap())
                    # one matmul: per-chunk sums broadcast on every partition
                    bc_ps = ps_early.tile([P, NUM_CHUNKS], dt.float32,
                                          tag="psc")
                    nc.tensor.matmul(bc_ps[:], ones_sq[:], acts8[:],
                                     start=True, stop=True)

                    # per-partition (redundant, identical) top-2 from PSUM
                    maxv = route.tile([P, NUM_CHUNKS], dt.float32)
                    maxi = route.tile([P, NUM_CHUNKS], dt.uint32)
                    nc.vector.max(maxv[:], bc_ps[:])
                    nc.vector.max_index(maxi[:], maxv[:], bc_ps[:])
                    maxi_f = route.tile([P, TOP_K], dt.float32)
                    nc.vector.tensor_copy(maxi_f[:], maxi[:, 0:TOP_K])

                    # offR[p, s] = sel_s*128 + p, fused + cast on write
                    offR = route.tile([P, TOP_K], dt.int32)
                    nc.vector.tensor_scalar(
                        offR[:], maxi_f[:], 128.0,
                        scalar2=C_Rf[:, 0:1],
                        op0=mybir.AluOpType.mult, op1=mybir.AluOpType.add)

            # ------------ gather selected hT (one row-gather per slot) -----
            g = [gath.tile([P, DT_ * BS], dt.bfloat16, tag=f"g{s}",
                           name=f"g{s}")
                 for s in range(TOP_K)]
            with tc.tile_wait_until(0.152):
                for s in range(TOP_K):
                    nc.gpsimd.indirect_dma_start(
                        out=g[s][:], out_offset=None,
                        in_=ht_d[:],
                        in_offset=bass.IndirectOffsetOnAxis(
                            ap=offR[:, s:s + 1], axis=0))

            with tc.tile_pool(name="ps_o", bufs=6, space="PSUM") as ps_o:
                # ------------ L2: out = h @ W_final + b_final --------------
                for o in range(OT):
                    osl = slice(o * 512, (o + 1) * 512)
                    for bt in range(BT):
                        po = ps_o.tile([P, 512], dt.float32, tag="po",
                                       name=f"po{o}_{bt}")
                        for kf in range(KF):
                            s, d = divmod(kf, DT_)
                            lsl = slice(d * BS + bt * P,
                                        d * BS + (bt + 1) * P)
                            nc.tensor.matmul(
                                po[:], g[s][:, lsl],
                                wf_t[kf][:, osl],
                                start=(kf == 0), stop=(kf == KF - 1))
                        ot_sb = outs.tile([P, 512], dt.bfloat16, tag="ot",
                                          name=f"ot{o}_{bt}")
                        nc.vector.tensor_tensor(
                            out=ot_sb[:], in0=po[:], in1=bfin_bc[:, osl],
                            op=mybir.AluOpType.add)
                        oeng = nc.sync if (o + bt) % 2 == 0 else nc.scalar
                        oeng.dma_start(
                            out[bt * P:(bt + 1) * P, osl], ot_sb[:])
    nc.compile()
    return nc


def _pack_table(a):
    # [8, 512, N] -> [1024, 4*N] with row (c*128+p) = a[c, kt*128+p, :] for
    # kt = 0..3 laid side by side
    n = a.shape[-1]
    return np.ascontiguousarray(
        a.reshape(NUM_CHUNKS, KT, P, n).transpose(0, 2, 1, 3)
        .reshape(NUM_CHUNKS * P, KT * n))


def kernel(x, W_chunks, b_chunks, W_final, b_final):
    bf16 = ml_dtypes.bfloat16
    x = np.asarray(x, dtype=np.float32).astype(bf16)
    W_chunks = np.asarray(W_chunks, dtype=np.float32).astype(bf16)
    W_final = np.asarray(W_final, dtype=np.float32).astype(bf16)
    b_chunks = np.ascontiguousarray(np.asarray(b_chunks, dtype=np.float32))
    b_final = np.ascontiguousarray(
        np.asarray(b_final, dtype=np.float32).reshape(1, OUT_F))

    wg = _pack_table(W_chunks)

    cmix = np.empty((P, 2), dtype=np.float32)
    cmix[:, :] = np.arange(P, dtype=np.float32)[:, None]

    if "nc" not in _cache:
        _cache["nc"] = _build()
    nc = _cache["nc"]

    in_maps = []
    for c in range(N_CORES):
        shard = x[c * BS:(c + 1) * BS]              # [512, 4096]
        xt = shard.T.reshape(NUM_CHUNKS, CIN, BS)   # [8, 512, 512]
        in_maps.append({
            "xg_shard": _pack_table(xt),
            "wg_chunks": wg,
            "b_chunks": b_chunks,
            "W_final": W_final,
            "b_final": b_final,
            "cmix": cmix,
        })

    res = run_bass_kernel_spmd(nc, in_maps, core_ids=list(range(N_CORES)))
    kernel.last_result = res
    return np.concatenate(
        [res.results[c]["out_shard"].astype(np.float32)
         for c in range(N_CORES)], axis=0)


kernel.last_result = None


# revision 48
# speedup vs baseline: 1.0660x; 1.0178x over previous
"""Trainium2 Bass kernel for BlockChunkedActivityRoutedNet.

Reference computation (B=4096, IN_F=4096, 8 chunks of 512, top-2 by mean|x|,
chunk-expert Linears 512->512, concat -> final Linear 1024->4096):

    xr = x.reshape(B, 8, 512)
    activities = mean(|xr|, axis=(0, 2))            # over the WHOLE batch
    i0, i1 = top2(activities)                        # descending
    h = concat(xr[:, i0] @ Wc[i0] + bc[i0], xr[:, i1] @ Wc[i1] + bc[i1])
    out = h @ W_final + b_final

Distribution: data-parallel over the batch across 8 NeuronCores (512 rows
each). Per-partition |x| partials are AllReduced ([128, 8] f32) so every
core computes identical top-2 routing; the partition reduce happens after
the collective.

The collective takes 65-95us wall (runtime barrier + mesh op + core start
skew) no matter when it is triggered, so the kernel fills that window with
SPECULATIVE work: L1 (x_c @ W_c + b_c) is computed for ALL 8 chunks while
the AllReduce runs, and the resulting hT tiles are streamed to a packed
DRAM table. After routing arrives, selecting a chunk is ONE indirect
row-gather ([128, 2048], 4KB lines), so the post-routing critical path is
just: top-2 + offsets (~3us) -> 2 gathers (~3us) -> L2 (~75us) -> tail.

Host-side prep inside kernel(): casts to bf16 and packs each core's x shard
and the chunk weights as [1024, 2048] tables

    xg[c*128 + p, kt*512 + b] = x.T[c*512 + kt*128 + p, b]
    wg[c*128 + p, kt*512 + d] = W_chunks[c, kt*128 + p, d]

so chunk c's tile is the static row-slice [c*128:(c+1)*128] with 4KB
contiguous lines per partition. hT_all uses the same layout
(ht[c*128 + p, d*512 + b] = hT[c][d][p, b]). W_final loads as 8
[128, 4096] bf16 tiles (8KB lines, full DMA rate). Output is written bf16
and upcast to f32 on the host.
"""

import numpy as np
import ml_dtypes

import concourse.bass as bass
import concourse.bacc as bacc
import concourse.mybir as mybir
from concourse.tile import TileContext
from concourse.bass_utils import run_bass_kernel_spmd
from concourse.masks import make_identity

dt = mybir.dt
P = 128

NUM_CHUNKS = 8
TOP_K = 2
IN_F = 4096
HID_F = 4096
OUT_F = 4096
B = 4096
CIN = IN_F // NUM_CHUNKS      # 512
COUT = HID_F // NUM_CHUNKS    # 512
N_CORES = 8
BS = B // N_CORES             # 512 rows per core

BT = BS // P                  # 4 batch tiles per core
KT = CIN // P                 # 4 k-tiles per selected chunk
DT_ = COUT // P               # 4 d-tiles per selected chunk
KF = TOP_K * DT_              # 8 k-tiles for the final matmul
OT = OUT_F // 512             # 8 output column tiles of 512
GW = KT * BS                  # 2048 packed-table row width

_cache = {}


def _build():
    nc = bacc.Bacc(num_devices=N_CORES, name="chunk_routed_net",
                   num_swdge_queues=4)

    xg = nc.dram_tensor("xg_shard", [NUM_CHUNKS * P, GW], dt.bfloat16,
                        kind="ExternalInput")
    wg = nc.dram_tensor("wg_chunks", [NUM_CHUNKS * P, GW], dt.bfloat16,
                        kind="ExternalInput")
    bc_t = nc.dram_tensor("b_chunks", [NUM_CHUNKS, COUT], dt.float32,
                          kind="ExternalInput")
    Wf = nc.dram_tensor("W_final", [COUT * TOP_K, OUT_F], dt.bfloat16,
                        kind="ExternalInput")
    bf = nc.dram_tensor("b_final", [1, OUT_F], dt.float32, kind="ExternalInput")
    cmix_d = nc.dram_tensor("cmix", [P, 2], dt.float32, kind="ExternalInput")
    out = nc.dram_tensor("out_shard", [BS, OUT_F], dt.bfloat16,
                         kind="ExternalOutput")

    # AllReduce buffers + speculative hT table
    cc_in = nc.dram_tensor("cc_in", [P, NUM_CHUNKS], dt.float32)
    cc_out = nc.dram_tensor("cc_out", [P, NUM_CHUNKS], dt.float32)
    ht_d = nc.dram_tensor("ht_all", [NUM_CHUNKS * P, DT_ * BS], dt.bfloat16)

    with TileContext(nc) as tc:
        with tc.tile_pool(name="consts", bufs=1) as consts, \
             tc.tile_pool(name="route", bufs=1) as route, \
             tc.tile_pool(name="xl", bufs=1) as xl_pool, \
             tc.tile_pool(name="wcp", bufs=1) as wc_pool, \
             tc.tile_pool(name="hta", bufs=4) as ht_pool, \
             tc.tile_pool(name="gath", bufs=1) as gath, \
             tc.tile_pool(name="bfinp", bufs=1) as bfinp, \
             tc.tile_pool(name="wfs", bufs=8) as wfs, \
             tc.tile_pool(name="outs", bufs=4) as outs:

            # ---------------- constants ----------------
            ones_col = consts.tile([P, 1], dt.float32)     # partition reduce
            nc.vector.memset(ones_col[:], 1.0)
            ones_k1 = consts.tile([1, P], dt.float32)      # K=1 bcast matmul
            nc.vector.memset(ones_k1[:], 1.0)
            ones_k1h = consts.tile([1, P], dt.bfloat16)    # K=1 bf16 bcast
            nc.vector.memset(ones_k1h[:], 1.0)
            ident = consts.tile([P, P], dt.float32)
            make_identity(nc, ident)
            ones_sq = consts.tile([P, P], dt.float32)  # bcast-sum matmul
            nc.vector.memset(ones_sq[:], 1.0)
            # C_Rf[p, :] = p (host-provided row iota)
            C_Rf = consts.tile([P, 2], dt.float32)
            nc.scalar.dma_start(C_Rf[:], cmix_d[:])

            with tc.tile_pool(name="ps_early", bufs=1, space="PSUM") as ps_early, \
                 tc.tile_pool(name="ps_h", bufs=2, space="PSUM") as ps_h:
                # ------------ activities (x split across two queues) -------
                actcol = route.tile([P, NUM_CHUNKS], dt.float32)
                scr = route.tile([P, GW], dt.bfloat16)  # ACT throwaway
                xls = []
                H = GW // 2
                for c in range(NUM_CHUNKS):
                    xlt = xl_pool.tile([P, GW], dt.bfloat16, tag=f"xl{c}",
                                       name=f"xl{c}")
                    nc.sync.dma_start(xlt[:, 0:H],
                                      xg[c * P:(c + 1) * P, 0:H])
                    nc.scalar.dma_start(xlt[:, H:GW],
                                        xg[c * P:(c + 1) * P, H:GW])
                    xls.append(xlt)
                for c in range(NUM_CHUNKS):
                    if c % 2 == 0:
                        nc.vector.tensor_reduce(
                            actcol[:, c:c + 1], xls[c][:],
                            axis=mybir.AxisListType.X, op=mybir.AluOpType.add,
                            apply_absolute_value=True)
                    else:
                        nc.scalar.activation(
                            scr[:], xls[c][:],
                            mybir.ActivationFunctionType.Abs,
                            accum_out=actcol[:, c:c + 1])

                # ------------ AllReduce of [128, 8] partials --------------
                nc.gpsimd.dma_start(cc_in.ap(), actcol[:])
                nc.gpsimd.collective_compute(
                    "AllReduce", mybir.AluOpType.add,
                    replica_groups=[list(range(N_CORES))],
                    ins=[cc_in.ap()], outs=[cc_out.ap()])

                # ---- work that fills the AllReduce window ----
                # biases first (tiny, must not queue behind the 8MB W_final:
                # the L1 evictions need bT)
                b_sb = route.tile([NUM_CHUNKS, COUT], dt.float32)
                nc.scalar.dma_start(b_sb[:], bc_t[:])
                bfin = bfinp.tile([1, OUT_F], dt.float32)
                nc.scalar.dma_start(bfin[:], bf[:])
                # b_chunks transpose: bT[p, d*8 + c] = bc[c, d*128 + p]
                bT = route.tile([P, DT_ * NUM_CHUNKS], dt.float32)
                for d in range(DT_):
                    ps_t = ps_early.tile([P, NUM_CHUNKS], dt.float32, tag="pst")
                    nc.tensor.transpose(
                        ps_t[:], b_sb[:, d * P:(d + 1) * P],
                        ident[0:NUM_CHUNKS, 0:NUM_CHUNKS])
                    nc.scalar.copy(bT[:, d * NUM_CHUNKS:(d + 1) * NUM_CHUNKS],
                                   ps_t[:])
                # b_final broadcast [128, 4096]
                bfin_h = bfinp.tile([1, OUT_F], dt.bfloat16)
                nc.vector.tensor_copy(bfin_h[:], bfin[:])
                bfin_bc = bfinp.tile([P, OUT_F], dt.float32)
                for o in range(OT):
                    sl = slice(o * 512, (o + 1) * 512)
                    ps_b = ps_early.tile([P, 512], dt.float32, tag="psb")
                    nc.tensor.matmul(ps_b[:], ones_k1h[:], bfin_h[:, sl],
                                     start=True, stop=True)
                    nc.vector.tensor_copy(bfin_bc[:, sl], ps_b[:])
                # chunk weights: all 8 chunk tiles, sync queue after x
                wc_t = []
                for c in range(NUM_CHUNKS):
                    w = wc_pool.tile([P, GW], dt.bfloat16, tag=f"wc{c}",
                                     name=f"wc{c}")
                    nc.sync.dma_start(w[:], wg[c * P:(c + 1) * P, :])
                    wc_t.append(w)
                # W_final: 8 x [128, 4096] bf16 (8KB lines), scalar queue
                wf_t = []
                for kf in range(KF):
                    w = wfs.tile([P, OUT_F], dt.bfloat16, tag="wf",
                                 name=f"wf{kf}")
                    nc.scalar.dma_start(w[:], Wf[kf * P:(kf + 1) * P, :])
                    wf_t.append(w)

                # ---- speculative L1 for ALL chunks, hT -> DRAM table ----
                for c in range(NUM_CHUNKS):
                    htc = ht_pool.tile([P, DT_ * BS], dt.bfloat16,
                                       tag="ht", name=f"ht{c}")
                    for d in range(DT_):
                        ph = ps_h.tile([P, BS], dt.float32, tag="ph",
                                       name=f"ph{c}_{d}")
                        for kt in range(KT):
                            base = kt * 512 + d * P
                            nc.tensor.matmul(
                                ph[:], wc_t[c][:, base:base + P],
                                xls[c][:, kt * 512:(kt + 1) * 512],
                                start=(kt == 0), stop=(kt == KT - 1))
                        dsl = slice(d * BS, (d + 1) * BS)
                        bias_col = bT[:, d * NUM_CHUNKS + c:
                                      d * NUM_CHUNKS + c + 1]
                        # alternate whole-tile evictions between DVE and
                        # ScalarE so consecutive PSUM groups pipeline
                        if (c + d) % 2 == 0:
                            nc.vector.tensor_scalar(
                                htc[:, dsl], ph[:], bias_col, scalar2=None,
                                op0=mybir.AluOpType.add)
                        else:
                            nc.scalar.activation(
                                htc[:, dsl], ph[:],
                                mybir.ActivationFunctionType.Identity,
                                bias=bias_col)
                        # sync queue only: gpsimd queues are held by the
                        # collective until it completes, and the scalar
                        # sequencer is the eviction pacer
                        nc.sync.dma_start(
                            ht_d[c * P:(c + 1) * P, dsl], htc[:, dsl])

                # ------------ routing (post-AllReduce) ------------
                # tile_wait_until fences the scheduler: everything here is
                # modeled as unavailable until ~70us sim-time, so the whole
                # speculative-L1 stream lands before it on every engine
                with tc.tile_wait_until(0.15):
                    acts8 = route.tile([P, NUM_CHUNKS], dt.float32)
                    nc.sync.dma_start(acts8[:], cc_out.ap())
                    # one matmul: per-chunk sums broadcast on every partition
                    bc_ps = ps_early.tile([P, NUM_CHUNKS], dt.float32,
                                          tag="psc")
                    nc.tensor.matmul(bc_ps[:], ones_sq[:], acts8[:],
                                     start=True, stop=True)

                    # per-partition (redundant, identical) top-2 from PSUM
                    maxv = route.tile([P, NUM_CHUNKS], dt.float32)
                    maxi = route.tile([P, NUM_CHUNKS], dt.uint32)
                    nc.vector.max(maxv[:], bc_ps[:])
                    nc.vector.max_index(maxi[:], maxv[:], bc_ps[:])
                    maxi_f = route.tile([P, TOP_K], dt.float32)
                    nc.vector.tensor_copy(maxi_f[:], maxi[:, 0:TOP_K])

                    # offR[p, s] = sel_s*128 + p, fused + cast on write
                    offR = route.tile([P, TOP_K], dt.int32)
                    nc.vector.tensor_scalar(
                        offR[:], maxi_f[:], 128.0,
                        scalar2=C_Rf[:, 0:1],
                        op0=mybir.AluOpType.mult, op1=mybir.AluOpType.add)

            # ------------ gather selected hT (one row-gather per slot) -----
            g = [gath.tile([P, DT_ * BS], dt.bfloat16, tag=f"g{s}",
                           name=f"g{s}")
                 for s in range(TOP_K)]
            with tc.tile_wait_until(0.152):
                for s in range(TOP_K):
                    nc.gpsimd.indirect_dma_start(
                        out=g[s][:], out_offset=None,
                        in_=ht_d[:],
                        in_offset=bass.IndirectOffsetOnAxis(
                            ap=offR[:, s:s + 1], axis=0))

            with tc.tile_pool(name="ps_o", bufs=6, space="PSUM") as ps_o:
                # ------------ L2: out = h @ W_final + b_final --------------
                for o in range(OT):
                    osl = slice(o * 512, (o + 1) * 512)
                    for bt in range(BT):
                        po = ps_o.tile([P, 512], dt.float32, tag="po",
                                       name=f"po{o}_{bt}")
                        for kf in range(KF):
                            s, d = divmod(kf, DT_)
                            lsl = slice(d * BS + bt * P,
                                        d * BS + (bt + 1) * P)
                            nc.tensor.matmul(
                                po[:], g[s][:, lsl],
                                wf_t[kf][:, osl],
                                start=(kf == 0), stop=(kf == KF - 1))
                        ot_sb = outs.tile([P, 512], dt.bfloat16, tag="ot",
                                          name=f"ot{o}_{bt}")
                        nc.vector.tensor_tensor(
                            out=ot_sb[:], in0=po[:], in1=bfin_bc[:, osl],
                            op=mybir.AluOpType.add)
                        oeng = nc.sync if (o + bt) % 2 == 0 else nc.scalar
                        oeng.dma_start(
                            out[bt * P:(bt + 1) * P, osl], ot_sb[:])
    nc.compile()
    return nc


def _pack_table(a):
    # [8, 512, N] -> [1024, 4*N] with row (c*128+p) = a[c, kt*128+p, :] for
    # kt = 0..3 laid side by side
    n = a.shape[-1]
    return np.ascontiguousarray(
        a.reshape(NUM_CHUNKS, KT, P, n).transpose(0, 2, 1, 3)
        .reshape(NUM_CHUNKS * P, KT * n))


def kernel(x, W_chunks, b_chunks, W_final, b_final):
    bf16 = ml_dtypes.bfloat16
    x = np.asarray(x, dtype=np.float32).astype(bf16)
    W_chunks = np.asarray(W_chunks, dtype=np.float32).astype(bf16)
    W_final = np.asarray(W_final, dtype=np.float32).astype(bf16)
    b_chunks = np.ascontiguousarray(np.asarray(b_chunks, dtype=np.float32))
    b_final = np.ascontiguousarray(
        np.asarray(b_final, dtype=np.float32).reshape(1, OUT_F))

    wg = _pack_table(W_chunks)

    cmix = np.empty((P, 2), dtype=np.float32)
    cmix[:, :] = np.arange(P, dtype=np.float32)[:, None]

    if "nc" not in _cache:
        _cache["nc"] = _build()
    nc = _cache["nc"]

    in_maps = []
    for c in range(N_CORES):
        shard = x[c * BS:(c + 1) * BS]              # [512, 4096]
        xt = shard.T.reshape(NUM_CHUNKS, CIN, BS)   # [8, 512, 512]
        in_maps.append({
            "xg_shard": _pack_table(xt),
            "wg_chunks": wg,
            "b_chunks": b_chunks,
            "W_final": W_final,
            "b_final": b_final,
            "cmix": cmix,
        })

    res = run_bass_kernel_spmd(nc, in_maps, core_ids=list(range(N_CORES)))
    kernel.last_result = res
    return np.concatenate(
        [res.results[c]["out_shard"].astype(np.float32)
         for c in range(N_CORES)], axis=0)


kernel.last_result = None
